# revision 10
# baseline (speedup 1.0000x reference)
"""Causal self-attention (B=4, S=2048, D=1024, H=16) on 8 TRN2 NeuronCores.

Sharding (tensor-parallel on heads + data-parallel on batch):
  core c -> batch c//2, head-half c%2 (8 of 16 heads).
  Wq/Wk/Wv column-split, Wo row-split; the two partial outputs per batch are
  summed on the host (+ bo), which is the row-parallel unshard.

Per-core Bass/Tile program (matmul operands bf16, psum/softmax fp32).
v4 design (trace-driven):
  - Attention phase is ACT(exp)-bound per key tile (one fused [128,2,SQ]
    ACTIVATE per tile vs PE ~0.85us), so projection / out-projection matmul
    chains are interleaved as "fillers" inside the attention loops: the PE
    FIFO always has exp-independent work while ACT grinds exps.
  - softmax normalization: partition-shifted copy of the sumexp row +
    reciprocal_approx_fast + gpsimd partition_broadcast (no PE broadcast
    matmul, no full-precision reciprocal).
  - causal masking: multiplicative triangular zeroing of bf16 probs on the
    otherwise-idle gpsimd engine (affine_select), off the DVE.
  - phase-A psum->sbuf copies with bias on DVE (tensor_scalar per-partition
    add), keeping the ACT queue pure exps during attention.
  - q/k/v projections split per superblock pair and scheduled so exp work
    starts as early as possible; input DMAs spread over idle engine queues.
"""

from contextlib import ExitStack

import numpy as np
import ml_dtypes

import concourse.bass as bass
import concourse.bacc as bacc
import concourse.tile as tile
import concourse.mybir as mybir

F32 = mybir.dt.float32
BF16 = mybir.dt.bfloat16


def build_core_program(S=2048, D=1024, HC=8, DH=64, SQ=512, mm_dt=BF16,
                       probs_bufs=6):
    """Build the per-core Bass program (SPMD: same program, different data).
    The host passes xT/wqk/wv/wo as bfloat16 with the 1/8 q-scale folded
    into the Wq columns of wqk."""
    DQ = HC * DH              # head-slice width (512)
    DK = D // 128             # contraction tiles for projections (8)
    DQN = DQ // 128           # head-pair tiles (4)
    NSB = S // SQ             # query superblocks (4)
    NTT = S // 128            # token tiles (16)
    NOUT = min(512, D)        # output-proj free width
    NOB = D // NOUT           # output-proj col blocks (2)
    ND = SQ // 128            # key tiles per superblock (4)
    assert DQ % 128 == 0 and S % SQ == 0 and SQ % 128 == 0 and D % 128 == 0

    nc = bacc.Bacc("TRN2", target_bir_lowering=False, debug=False)

    xT = nc.dram_tensor("xT", [D, S], mm_dt, kind="ExternalInput").ap()
    wqk = nc.dram_tensor("wqk", [D, 2 * DQ], mm_dt, kind="ExternalInput").ap()
    wv = nc.dram_tensor("wv", [D, DQ], mm_dt, kind="ExternalInput").ap()
    wo = nc.dram_tensor("wo", [DQ, D], mm_dt, kind="ExternalInput").ap()
    bqk = nc.dram_tensor("bqk", [2 * DQ], F32, kind="ExternalInput").ap()
    bv = nc.dram_tensor("bv", [DQ], F32, kind="ExternalInput").ap()
    out = nc.dram_tensor("out", [S, D], F32, kind="ExternalOutput").ap()

    with tile.TileContext(nc) as tc, ExitStack() as ctx:
        ctx.enter_context(nc.allow_low_precision(
            reason="low-precision matmul operands; accumulation stays fp32"))
        const = ctx.enter_context(tc.tile_pool(name="const", bufs=1))
        big = ctx.enter_context(tc.tile_pool(name="big", bufs=1))
        stream = ctx.enter_context(tc.tile_pool(name="stream", bufs=1))
        psum = ctx.enter_context(tc.tile_pool(name="psum", bufs=1, space="PSUM"))

        # biases: bqk as [128, 2*DQN] (column t = dout tile t), bv broadcast
        bqk_sb = const.tile([128, 2 * DQN], F32)
        nc.sync.dma_start(bqk_sb[:], bqk.rearrange("(t p) -> p t", p=128))
        bv_rowf = const.tile([1, DQ], F32)
        nc.sync.dma_start(bv_rowf[:], bv.rearrange("(a d) -> a d", a=1))
        bv_bc = const.tile([128, DQ], F32)
        nc.gpsimd.partition_broadcast(bv_bc[:], bv_rowf[:])

        # ---- big resident tensors ----
        kT = big.tile([128, DQN, S], mm_dt)     # [pair 2x64 rows, tokens]
        qT = big.tile([128, DQN, S], mm_dt)
        v_aug = big.tile([128, NTT, HC * 65], mm_dt)
        wqk_sb = big.tile([128, DK, 2 * DQ], mm_dt)
        wv_sb = big.tile([128, DK, DQ], mm_dt)
        wo_sb = big.tile([128, DQN, D], mm_dt)
        xt_all = big.tile([128, DK, S], mm_dt)

        # Priority-ordered input loading (only SP/ACT/gpsimd can start DMAs):
        # x token-halves on SP/ACT queues (first projection chain reads
        # tokens 0-1023 first), wqk in per-dout column blocks on gpsimd in
        # the order the projection chains consume them (dt 0 and DQN first).
        for h in range(2):
            q = [nc.sync, nc.scalar][h]
            for kt in range(DK):
                q.dma_start(xt_all[:, kt, h * S // 2:(h + 1) * S // 2],
                            xT[128 * kt:128 * (kt + 1),
                               h * S // 2:(h + 1) * S // 2])
        dt_order = [0, DQN]
        for k in range(1, DQN):
            dt_order += [k, DQN + k]
        for dt in dt_order:
            for kq in range(2):
                src = wqk[512 * kq:512 * (kq + 1), 128 * dt:128 * (dt + 1)]
                nc.gpsimd.dma_start(
                    wqk_sb[:, 4 * kq:4 * kq + 4, 128 * dt:128 * (dt + 1)],
                    src.rearrange("(a p) d -> p a d", p=128))
        for kt in range(DK):
            nc.gpsimd.dma_start(wv_sb[:, kt, :], wv[128 * kt:128 * (kt + 1), :])
        for p4 in range(DQN):
            nc.gpsimd.dma_start(wo_sb[:, p4, :], wo[128 * p4:128 * (p4 + 1), :])

        # ones column of v_aug (column 64 of each head slot), set once
        va4 = v_aug[:].rearrange("p t (h c) -> p t h c", h=HC)
        nc.vector.memset(va4[:, :, :, 64:65], 1.0)

        def proj_qk_unit(tbp, dt):
            """q/k projection chain: one dout tile, superblock pair tbp."""
            pss = psum.tile([128, 2, SQ], F32, tag="sc", bufs=2,
                            name=f"pss_{tbp}_{dt}")
            for h in range(2):
                # h-major so the chain starts as soon as the first x
                # token-half has landed
                tb = 2 * tbp + h
                for kt in range(DK):
                    nc.tensor.matmul(
                        pss[:, h, :],
                        wqk_sb[:, kt, 128 * dt:128 * (dt + 1)],
                        xt_all[:, kt, tb * SQ:(tb + 1) * SQ],
                        start=(kt == 0), stop=(kt == DK - 1))
            dest = qT if dt < DQN else kT
            hp = dt % DQN
            nc.vector.tensor_scalar(
                out=dest[:, hp, 2 * tbp * SQ:2 * (tbp + 1) * SQ],
                in0=pss[:].rearrange("p a b -> p (a b)"),
                scalar1=bqk_sb[:, dt:dt + 1], scalar2=None,
                op0=mybir.AluOpType.add)

        def proj_v_unit(tt):
            # v projection for one token tile, bias added on DVE
            psv = psum.tile([128, DQ], F32, tag="out", bufs=2,
                            name=f"psv_{tt}")
            for kt in range(DK):
                nc.tensor.matmul(
                    psv[:], xt_all[:, kt, 128 * tt:128 * (tt + 1)],
                    wv_sb[:, kt, :],
                    start=(kt == 0), stop=(kt == DK - 1))
            va = v_aug[:, tt, :].rearrange("p (h c) -> p h c", h=HC)
            nc.vector.tensor_tensor(
                va[:, :, 0:64], psv[:].rearrange("p (h c) -> p h c", h=HC),
                bv_bc[:].rearrange("p (h c) -> p h c", h=HC),
                op=mybir.AluOpType.add)

        def out_unit(i, attnT, mm_):
            # out-projection for one token tile of superblock i
            tt = i * ND + mm_
            pos = [psum.tile([128, NOUT], F32, tag="out", bufs=2,
                             name=f"po_{tt}_{nb}") for nb in range(NOB)]
            for p4 in range(DQN):
                for nb in range(NOB):
                    nc.tensor.matmul(
                        pos[nb][:],
                        attnT[:, p4, 128 * mm_:128 * (mm_ + 1)],
                        wo_sb[:, p4, nb * NOUT:(nb + 1) * NOUT],
                        start=(p4 == 0), stop=(p4 == DQN - 1))
            for nb in range(NOB):
                osb = stream.tile([128, NOUT], F32, tag="osb", bufs=3,
                                  name=f"ob_{tt}_{nb}")
                nc.vector.tensor_copy(osb[:], pos[nb][:])
                nc.sync.dma_start(
                    out[128 * tt:128 * (tt + 1),
                        nb * NOUT:(nb + 1) * NOUT], osb[:])

        def attention(i, attnT, fillers):
            """scores/softmax/PV for query superblock i -> attnT (bf16).
            fillers: exp-independent PE work drained inside the loops."""
            fillers = list(fillers)

            def drain():
                if fillers:
                    fillers.pop(0)()

            NJ = ND * (i + 1)
            dp = sorted({max(1, NJ // 3), max(2, (2 * NJ) // 3)})
            for hp in range(DQN):
                pva = psum.tile([65, SQ], F32, tag="pv", bufs=2,
                                name=f"pv_{i}_{hp}_0")
                pvb = psum.tile([65, SQ], F32, tag="pv", bufs=2,
                                name=f"pv_{i}_{hp}_1")
                pvs = (pva, pvb)
                pend = None
                for j in range(NJ):
                    jj = j - ND * i
                    f0 = max(0, 128 * jj)
                    sc = psum.tile([128, 2, SQ], F32, tag="sc", bufs=2,
                                   name=f"sc_{i}_{hp}_{j}")
                    for hh in range(2):
                        p0, p1 = 64 * hh, 64 * hh + 64
                        nc.tensor.matmul(
                            sc[:, hh, f0:],
                            kT[p0:p1, hp, 128 * j:128 * (j + 1)],
                            qT[p0:p1, hp, i * SQ + f0:(i + 1) * SQ],
                            start=True, stop=True,
                            tile_position=(64 * hh, 0))
                    probs = stream.tile([128, 2, SQ], mm_dt, tag="probs",
                                        bufs=probs_bufs,
                                        name=f"pr_{i}_{hp}_{j}")
                    nc.scalar.activation(
                        probs[:, :, f0:], sc[:, :, f0:],
                        mybir.ActivationFunctionType.Exp)
                    if jj >= 0:
                        # diagonal boundary tile: zero probs above the
                        # diagonal (gpsimd, keeps DVE off the exp->PV path)
                        for hh in range(2):
                            nc.gpsimd.affine_select(
                                out=probs[:, hh, f0:f0 + 128],
                                in_=probs[:, hh, f0:f0 + 128],
                                compare_op=mybir.AluOpType.is_ge,
                                fill=0.0, base=0, channel_multiplier=-1,
                                pattern=[[1, 128]])
                    if pend is not None:
                        pprbs, pf0, pj = pend
                        for hh in range(2):
                            h = 2 * hp + hh
                            nc.tensor.matmul(
                                pvs[hh][:, pf0:],
                                v_aug[:, pj, 65 * h:65 * h + 65],
                                pprbs[:, hh, pf0:],
                                start=(pj == 0), stop=(pj == NJ - 1))
                    pend = (probs, f0, j)
                    if j in dp:
                        drain()
                pprbs, pf0, pj = pend
                for hh in range(2):
                    h = 2 * hp + hh
                    nc.tensor.matmul(
                        pvs[hh][:, pf0:],
                        v_aug[:, pj, 65 * h:65 * h + 65],
                        pprbs[:, hh, pf0:],
                        start=(pj == 0), stop=(pj == NJ - 1))
                # normalization: shifted copy of the sumexp row, fast recip,
                # gpsimd broadcast, per-column scale
                for hh in range(2):
                    srow = stream.tile([1, SQ], F32, tag="srow", bufs=4,
                                       name=f"sr_{i}_{hp}_{hh}")
                    nc.vector.tensor_copy(srow[:], pvs[hh][64:65, :])
                    rc = stream.tile([1, SQ], F32, tag="rc", bufs=4,
                                     name=f"rc_{i}_{hp}_{hh}")
                    nc.vector.reciprocal_approx_fast(out=rc[:], in_=srow[:])
                    bc = stream.tile([64, SQ], F32, tag="bc", bufs=4,
                                     name=f"bc_{i}_{hp}_{hh}")
                    nc.gpsimd.partition_broadcast(bc[:], rc[:])
                    if hh == 0:
                        nc.vector.tensor_tensor(
                            attnT[0:64, hp, :], pvs[hh][0:64, :], bc[:],
                            op=mybir.AluOpType.mult)
                    else:
                        stage = stream.tile([64, SQ], mm_dt, tag="stage",
                                            bufs=2, name=f"st_{i}_{hp}")
                        nc.vector.tensor_tensor(
                            stage[:], pvs[hh][0:64, :], bc[:],
                            op=mybir.AluOpType.mult)
                        nc.sync.dma_start(attnT[64:128, hp, :], stage[:])
                drain()
            while fillers:
                fillers.pop(0)()

        # all four attnT tiles stay live: out-proj fillers for superblock i
        # run inside attention(i+2), so no slot reuse is safe
        ats = [stream.tile([128, DQN, SQ], mm_dt, tag="attnT", bufs=NSB,
                           name=f"at_{i}") for i in range(NSB)]
        mk = lambda f, *a: (lambda: f(*a))

        # q/k for superblock-pair 0, head-pair 0 + v tiles 0-3 upfront so
        # attention(0) head-pair 0 can start immediately; everything else
        # drains as fillers inside the ACT-bound attention loops.
        P = lambda: None  # padding: spreads fillers across drain points
        proj_qk_unit(0, 0)
        proj_qk_unit(0, DQN)
        for tt in range(ND):
            proj_v_unit(tt)
        attention(0, ats[0], [
            mk(proj_qk_unit, 0, 1), mk(proj_qk_unit, 0, DQN + 1),
            mk(proj_qk_unit, 0, 2), mk(proj_qk_unit, 0, DQN + 2),
            mk(proj_qk_unit, 0, 3), mk(proj_qk_unit, 0, DQN + 3),
            mk(proj_v_unit, ND), mk(proj_v_unit, ND + 1),
            mk(proj_v_unit, ND + 2), mk(proj_v_unit, ND + 3),
        ])
        attention(1, ats[1], [
            mk(proj_qk_unit, 1, 0), mk(proj_qk_unit, 1, DQN),
            mk(proj_qk_unit, 1, 1), mk(proj_qk_unit, 1, DQN + 1),
            mk(proj_qk_unit, 1, 2), mk(proj_qk_unit, 1, DQN + 2),
            mk(proj_qk_unit, 1, 3), mk(proj_qk_unit, 1, DQN + 3),
            mk(proj_v_unit, 2 * ND), mk(proj_v_unit, 2 * ND + 1),
            mk(proj_v_unit, 2 * ND + 2), mk(proj_v_unit, 2 * ND + 3),
        ])
        attention(2, ats[2], [
            mk(proj_v_unit, 3 * ND), mk(proj_v_unit, 3 * ND + 1),
            mk(proj_v_unit, 3 * ND + 2), mk(proj_v_unit, 3 * ND + 3),
            mk(out_unit, 0, ats[0], 0), mk(out_unit, 0, ats[0], 1),
            mk(out_unit, 0, ats[0], 2), mk(out_unit, 0, ats[0], 3),
            mk(out_unit, 1, ats[1], 0), mk(out_unit, 1, ats[1], 1),
            mk(out_unit, 1, ats[1], 2), mk(out_unit, 1, ats[1], 3),
        ])
        attention(3, ats[3], [
            mk(out_unit, 2, ats[2], 0), P, mk(out_unit, 2, ats[2], 1), P,
            mk(out_unit, 2, ats[2], 2), P, mk(out_unit, 2, ats[2], 3), P,
        ])

        # final out-projection: two-bank pos pairs in the now-free "sc"
        # slots; each tile's head-pair<3 matmuls are emitted before any
        # head-pair-3 matmul so the PE has runnable work while the last
        # head-pair's normalize + partition-shift DMA completes
        def c3_head(mm_):
            pp = psum.tile([128, NOB, NOUT], F32, tag="sc", bufs=2,
                           name=f"pp_{mm_}")
            for p4 in range(DQN - 1):
                for nb in range(NOB):
                    nc.tensor.matmul(
                        pp[:, nb, :],
                        ats[3][:, p4, 128 * mm_:128 * (mm_ + 1)],
                        wo_sb[:, p4, nb * NOUT:(nb + 1) * NOUT],
                        start=(p4 == 0), stop=False)
            return pp

        def c3_tail(mm_, pp):
            tt = 3 * ND + mm_
            for nb in range(NOB):
                nc.tensor.matmul(
                    pp[:, nb, :],
                    ats[3][:, DQN - 1, 128 * mm_:128 * (mm_ + 1)],
                    wo_sb[:, DQN - 1, nb * NOUT:(nb + 1) * NOUT],
                    start=False, stop=True)
            osb = stream.tile([128, NOB * NOUT], F32, tag="osb3", bufs=2,
                              name=f"ob3_{mm_}")
            nc.vector.tensor_copy(osb[:], pp[:].rearrange("p a b -> p (a b)"))
            nc.sync.dma_start(out[128 * tt:128 * (tt + 1), :], osb[:])

        pp0 = c3_head(0)
        pp1 = c3_head(1)
        c3_tail(0, pp0)
        pp2 = c3_head(2)
        c3_tail(1, pp1)
        pp3 = c3_head(3)
        c3_tail(2, pp2)
        c3_tail(3, pp3)

    nc.compile()
    return nc

B, S, D, H = 4, 2048, 1024, 16
N_CORES = 8

_CACHED = {}


def _make_core_inputs(x, Wq, bq, Wk, bk, Wv, bv, Wo):
    DQ = D // 2

    def cast(a):
        return np.ascontiguousarray(a).astype(ml_dtypes.bfloat16)

    xTs = [cast(x[b].T) for b in range(B)]
    in_maps = []
    for c in range(N_CORES):
        b, hf = c // 2, c % 2
        sl = slice(hf * DQ, (hf + 1) * DQ)
        in_maps.append({
            "xT": xTs[b],
            "wqk": cast(np.concatenate([0.125 * Wq[:, sl], Wk[:, sl]],
                                       axis=1)),
            "wv": cast(Wv[:, sl]),
            "wo": cast(Wo[sl, :]),
            "bqk": np.ascontiguousarray(
                np.concatenate([0.125 * bq[sl], bk[sl]])).astype(np.float32),
            "bv": np.ascontiguousarray(bv[sl]).astype(np.float32),
        })
    return in_maps


def kernel(x, Wq, bq, Wk, bk, Wv, bv, Wo, bo):
    import tempfile
    from concourse import bass_utils

    x = np.asarray(x, dtype=np.float32)
    Wq = np.asarray(Wq, dtype=np.float32)
    bq = np.asarray(bq, dtype=np.float32)
    Wk = np.asarray(Wk, dtype=np.float32)
    bk = np.asarray(bk, dtype=np.float32)
    Wv = np.asarray(Wv, dtype=np.float32)
    bv = np.asarray(bv, dtype=np.float32)
    Wo = np.asarray(Wo, dtype=np.float32)
    bo = np.asarray(bo, dtype=np.float32)

    if "nc" not in _CACHED:
        _CACHED["nc"] = build_core_program(S=S, D=D, HC=H // 2)
    nc = _CACHED["nc"]

    in_maps = _make_core_inputs(x, Wq, bq, Wk, bk, Wv, bv, Wo)
    res = bass_utils.run_bass_kernel_spmd(
        nc, in_maps, core_ids=list(range(N_CORES)),
        tmpdir=tempfile.mkdtemp(prefix="bass_attn_"))

    out = np.empty((B, S, D), dtype=np.float32)
    for b in range(B):
        out[b] = res.results[2 * b]["out"] + res.results[2 * b + 1]["out"] + bo
    return out
